# revision 40
# baseline (speedup 1.0000x reference)
"""Single-head causal attention (B=8, T=2048, C=1024, H=64) on 8 TRN2 NeuronCores.

Strategy (data-parallel over batch, one batch element per core):
  - Host transposes x[b] -> xT [C, T], casts matmul operands to bf16, and
    prepacks all weights/constants into one SBUF-layout buffer.  DMA order
    is by first use (wqk chunk 0, xt tile 0, remaining wqk, ...) so the
    first proj matmul starts ~2us earlier than with per-tensor DMAs.
  - Device, per core, per 512-wide t-block tb:
      proj(tb):  qT,kT = ([Wq|Wk].T @ xT_tb) packed in one PE pass; vT = Wv.T @ xT_tb
      evac(tb):  PSUM -> SBUF bf16 casts (kT via 64->0 partition-shift DVE copy)
      trans(tb): v chunks rebuilt in natural [s, h] layout via PE transpose,
                 with ONE ones-column appended (vext, [128, 65]) so the PV
                 matmul also produces the softmax denominator l in row 64.
                 65 output partitions instead of 128 halve the PE array
                 energy of the PV pass (the chip power-throttles the PE when
                 sustained power is too high, so energy == time; GpSimd is
                 kept idle for the same reason).
      attn(tb), per s-chunk pair (causally trimmed, exact packed widths):
          ST[s, t] = kT_chunk.T @ qT_block            (PSUM, <=2 banks/pair)
          diag chunks: += causal additive mask on first 128 cols (DVE)
          PT = exp(SCALE * ST)                        (one ACT per pair, bf16 out)
          PV[:, t] += vext_chunk.T @ PT               (rows 0-63 = out.T, 64 = l)
      epilogue, per 256-col half (starts before the block's last PV):
          DVE copies pv[0:65] -> bf16 SBUF, DMA to DRAM.
  - Host computes out = (pv_rows / l_row).T — the final normalize is part of
    the unshard/gather step (1M flops vs 17 GFLOP on device).
All matmul accumulation is fp32 (PSUM); bf16 operands give ~4e-3 l2 rel err.
"""

import numpy as np
import ml_dtypes
from contextlib import ExitStack

import concourse.bass as bass
from concourse import bacc
import concourse.mybir as mybir
import concourse.tile as tile
from concourse.bass import ts
from concourse.bass_utils import run_bass_kernel_spmd


B, T, C, H = 8, 2048, 1024, 64
P = 128
W_BLK = 512
HB = W_BLK // 2         # epilogue half-block
N_TB = T // W_BLK       # 4 t-blocks
N_C = C // P            # 8 contraction chunks
N_S = T // P            # 16 s-chunks
N_J = W_BLK // P        # 4 diagonal chunks per t-block
SCALE = float(H) ** -0.5
NEG = -1e30
HL = H + 1              # PV output rows: 64 out dims + 1 denominator row

MM_DT = mybir.dt.bfloat16
NP_MM = ml_dtypes.bfloat16
F32 = mybir.dt.float32

# consts tile layout (bf16 columns): [wqk_c | wv_c] per c-chunk, then ident.
# Per-chunk interleave so one small leading DMA covers the first proj chunks.
CH_W = 3 * H            # 192 cols per c-chunk (128 wqk + 64 wv)
ID_OFF = N_C * CH_W     # 1536
CONST_W = ID_OFF + H    # 1600


def build_nc() -> bacc.Bacc:
    nc = bacc.Bacc("TRN2")
    consts_d = nc.dram_tensor("consts", [P, CONST_W], MM_DT, kind="ExternalInput")
    cmask_d = nc.dram_tensor("cmask", [P, P], F32, kind="ExternalInput")
    # host pre-tiles xT so each [128, 512] tile is one contiguous 128KB read
    xT_d = nc.dram_tensor("xT", [N_TB, N_C, P, W_BLK], MM_DT, kind="ExternalInput")
    # unnormalized out rows 0-63 + denominator row 64, per t-block halves
    out_d = nc.dram_tensor("out", [HL, T], MM_DT, kind="ExternalOutput")

    with tile.TileContext(nc) as tc, ExitStack() as ctx:
        const = ctx.enter_context(tc.tile_pool(name="const", bufs=1))

        consts = const.tile([P, CONST_W], MM_DT)

        def wqk_c(c):
            return consts[:, c * CH_W: c * CH_W + 2 * H]

        def wv_c(c):
            return consts[:, c * CH_W + 2 * H: (c + 1) * CH_W]

        ident = consts[0:H, ID_OFF: ID_OFF + H]

        xt = {}

        def load_xt_pair(tb, c):
            # one DMA per 2 c-chunks: descriptor-gen on the SP queue
            # (~0.65us/instr) limits the feed rate, not the transfer
            t_ = const.tile([P, 2, W_BLK], MM_DT, name=f"xt{c}_{tb}")
            nc.sync.dma_start(
                t_, xT_d[tb, c:c + 2].rearrange("o p t -> p o t"))
            xt[(c, tb)] = t_[:, 0, :]
            xt[(c + 1, tb)] = t_[:, 1, :]

        # DMA order = first-use order: weight chunks 0-3 first, then the
        # first xt pair, then the rest
        nc.sync.dma_start(consts[:, 0:4 * CH_W], consts_d[:, 0:4 * CH_W])
        load_xt_pair(0, 0)
        nc.sync.dma_start(consts[:, 4 * CH_W:CONST_W],
                          consts_d[:, 4 * CH_W:CONST_W])
        load_xt_pair(0, 2)
        cmask = const.tile([P, P], F32)
        nc.sync.dma_start(cmask, cmask_d[:])
        load_xt_pair(0, 4)
        load_xt_pair(0, 6)
        for tb in range(1, N_TB):
            for c in range(0, N_C, 2):
                load_xt_pair(tb, c)

        qT_blk = [const.tile([H, W_BLK], MM_DT, name=f"qT{tb}") for tb in range(N_TB)]
        kT_blk = [const.tile([H, W_BLK], MM_DT, name=f"kT{tb}") for tb in range(N_TB)]
        vT_blk = [const.tile([H, W_BLK], MM_DT, name=f"vT{tb}") for tb in range(N_TB)]
        # vext[s] = [v_nat(s) | ones-column]: PV yields out.T rows + l row
        vext = const.tile([P, N_S, HL], MM_DT, name="vext")
        nc.vector.memset(vext[:, :, H:HL], 1.0)

        with tc.tile_pool(name="ps_qk", bufs=1, space="PSUM") as ps_qk, \
             tc.tile_pool(name="ps_v", bufs=1, space="PSUM") as ps_v, \
             tc.tile_pool(name="ps_st", bufs=5, space="PSUM") as ps_st, \
             tc.tile_pool(name="ps_pv", bufs=1, space="PSUM") as ps_pv, \
             tc.tile_pool(name="ptp", bufs=7) as pt_pool, \
             tc.tile_pool(name="outp", bufs=4) as out_pool:

            # warm up the PE p-state during the DMA head: ~8 dummy matmuls on
            # garbage SBUF keep the PE continuously busy so the DVFS ramp
            # reaches full clock before the first real projection matmul.
            # Results land in st-pool tiles that real STs later overwrite
            # (start=True); the pool rotation orders them safely.
            warm_sb = const.tile([P, W_BLK], MM_DT, name="warm_sb")
            nc.vector.memset(warm_sb, 1.0)
            for wi in range(2):
                wtile = ps_st.tile([P, W_BLK], F32, tag="st", name=f"warm{wi}")
                for _ in range(4):
                    nc.tensor.matmul(wtile, warm_sb[:, 0:P], warm_sb[:],
                                     start=True, stop=True,
                                     skip_group_check=True)

            for tb in range(N_TB):
                # ---- proj(tb) ----
                qk_ps = ps_qk.tile([P, W_BLK], F32, tag="qk", name=f"qk{tb}")
                v_ps = ps_v.tile([H, W_BLK], F32, tag="v", name=f"v{tb}")
                # interleaved per c-chunk: each arriving xt tile feeds two
                # matmuls immediately, so the DMA stream keeps ahead of the PE
                for c in range(N_C):
                    nc.tensor.matmul(qk_ps, wqk_c(c), xt[(c, tb)],
                                     start=(c == 0), stop=(c == N_C - 1))
                    nc.tensor.matmul(v_ps, wv_c(c), xt[(c, tb)],
                                     start=(c == 0), stop=(c == N_C - 1))
                # vT first: the transpose chain (tr -> vext -> diag PV) is
                # longer than the qT/kT -> ST one; kT still lands before the
                # diagonal STs (5th+ in the ST stream) need it
                nc.vector.tensor_copy(vT_blk[tb][:], v_ps[:, :])
                nc.vector.tensor_copy(qT_blk[tb][:], qk_ps[0:H, :])
                # partition shift 64->0 (64-lane DVE op, quadrant-aligned)
                nc.vector.tensor_copy(kT_blk[tb][:], qk_ps[H:P, :])

                # ---- v transposes for this block (shares the qk psum tag) ----
                for j in range(N_J):
                    s = tb * N_J + j
                    tr = ps_qk.tile([P, H], MM_DT, tag="qk", name=f"tr{s}")
                    nc.tensor.transpose(tr, vT_blk[tb][:, ts(j, P)], ident)
                    nc.vector.tensor_copy(vext[:, s, 0:H], tr)

                # ---- attn(tb) ----
                pv = ps_pv.tile([HL, W_BLK], F32, tag="pv", name=f"pv{tb}")
                n_full = tb * N_J
                # (s_chunk, col offset within t-block, width)
                chunks = [(s, 0, W_BLK) for s in range(n_full)]
                chunks += [(n_full + j, j * P, W_BLK - j * P) for j in range(N_J)]
                n_ch = len(chunks)

                def emit_epi(half):
                    t0 = half * HB
                    ot = out_pool.tile([HL, HB], MM_DT, tag=f"ot{half}",
                                       name=f"ot{tb}_{half}")
                    nc.vector.tensor_copy(ot, pv[:, t0:t0 + HB])
                    nc.sync.dma_start(
                        out_d[:, tb * W_BLK + t0: tb * W_BLK + t0 + HB], ot)

                # single-chunk ST tiles (1 PSUM bank each) with lookahead-3
                # emission: PV(i) reaches the PE queue head only after the
                # exp(i) result is already in SBUF, so the PE pipeline never
                # drains on the exp dependency.
                pt_t = [None] * n_ch

                def emit_st(ci):
                    s, off, w = chunks[ci]
                    st_t = ps_st.tile([P, w], F32, tag="st", name=f"st{tb}_{ci}")
                    nc.tensor.matmul(st_t,
                                     kT_blk[s // N_J][:, ts(s % N_J, P)],
                                     qT_blk[tb][:, off:W_BLK],
                                     start=True, stop=True)
                    if s >= n_full:  # diagonal: mask first 128 cols
                        nc.vector.tensor_tensor(st_t[:, 0:P], st_t[:, 0:P],
                                                cmask, mybir.AluOpType.add)
                    pt = pt_pool.tile([P, w], MM_DT, tag="pt",
                                      name=f"pt{tb}_{ci}")
                    nc.scalar.activation(pt, st_t,
                                         mybir.ActivationFunctionType.Exp,
                                         scale=SCALE)
                    pt_t[ci] = pt

                def emit_pv(ci):
                    s, off, w = chunks[ci]
                    nc.tensor.matmul(pv[:, off:W_BLK], vext[:, s, :],
                                     pt_t[ci],
                                     start=(ci == 0), stop=(ci == n_ch - 1))

                LA = 4
                for ci in range(min(LA, n_ch)):
                    emit_st(ci)
                for ci in range(n_ch):
                    if ci + LA < n_ch:
                        emit_st(ci + LA)
                    emit_pv(ci)
                    # cols [0:256] are final once diag chunk j=1's PV is in
                    if ci == n_ch - 3:
                        emit_epi(0)
                emit_epi(1)

    nc.compile()
    return nc


_NC_CACHE = None


def _get_nc():
    global _NC_CACHE
    if _NC_CACHE is None:
        _NC_CACHE = build_nc()
    return _NC_CACHE


def prepare_in_maps(x, Wk, Wq, Wv):
    wqk = np.concatenate([np.asarray(Wq), np.asarray(Wk)], axis=1).astype(NP_MM)
    wv = np.asarray(Wv).astype(NP_MM)
    consts = np.zeros((P, CONST_W), dtype=NP_MM)
    wqk3 = wqk.reshape(N_C, P, 2 * H)
    wv3 = wv.reshape(N_C, P, H)
    for c in range(N_C):
        consts[:, c * CH_W: c * CH_W + 2 * H] = wqk3[c]
        consts[:, c * CH_W + 2 * H: (c + 1) * CH_W] = wv3[c]
    consts[0:H, ID_OFF:ID_OFF + H] = np.eye(H, dtype=NP_MM)
    # cmask[s, t] = 0 if t >= s else NEG (additive causal mask for diag chunks)
    ii = np.arange(P)
    cmask = np.where(ii[None, :] >= ii[:, None], 0.0, NEG).astype(np.float32)
    in_maps = []
    for b in range(B):
        xTb = np.asarray(x[b]).T.astype(NP_MM)  # [C, T]
        xT = np.ascontiguousarray(
            xTb.reshape(N_C, P, N_TB, W_BLK).transpose(2, 0, 1, 3)
        )  # [N_TB, N_C, 128, 512], each tile contiguous
        in_maps.append({"xT": xT, "consts": consts, "cmask": cmask})
    return in_maps


def run(x, Wk, Wq, Wv, trace=False):
    nc = _get_nc()
    in_maps = prepare_in_maps(x, Wk, Wq, Wv)
    res = run_bass_kernel_spmd(nc, in_maps, core_ids=list(range(B)), trace=trace)
    outs = []
    for r in res.results:
        o = np.asarray(r["out"], dtype=np.float32)  # [65, T]
        outs.append((o[0:H, :] / o[H:HL, :]).T)     # normalize + transpose
    return np.stack(outs), res


def kernel(x, Wk, Wq, Wv):
    out, _ = run(x, Wk, Wq, Wv, trace=False)
    return out


# revision 41
# speedup vs baseline: 1.0068x; 1.0068x over previous
"""Single-head causal attention (B=8, T=2048, C=1024, H=64) on 8 TRN2 NeuronCores.

Strategy (data-parallel over batch, one batch element per core):
  - Host transposes x[b] -> xT [C, T], casts matmul operands to bf16, and
    prepacks all weights/constants into one SBUF-layout buffer.  DMA order
    is by first use (wqk chunk 0, xt tile 0, remaining wqk, ...) so the
    first proj matmul starts ~2us earlier than with per-tensor DMAs.
  - Device, per core, per 512-wide t-block tb:
      proj(tb):  qT,kT = ([Wq|Wk].T @ xT_tb) packed in one PE pass; vT = Wv.T @ xT_tb
      evac(tb):  PSUM -> SBUF bf16 casts (kT via 64->0 partition-shift DVE copy)
      trans(tb): v chunks rebuilt in natural [s, h] layout via PE transpose,
                 with ONE ones-column appended (vext, [128, 65]) so the PV
                 matmul also produces the softmax denominator l in row 64.
                 65 output partitions instead of 128 halve the PE array
                 energy of the PV pass (the chip power-throttles the PE when
                 sustained power is too high, so energy == time; GpSimd is
                 kept idle for the same reason).
      attn(tb), per s-chunk pair (causally trimmed, exact packed widths):
          ST[s, t] = kT_chunk.T @ qT_block            (PSUM, <=2 banks/pair)
          diag chunks: += causal additive mask on first 128 cols (DVE)
          PT = exp(SCALE * ST)                        (one ACT per pair, bf16 out)
          PV[:, t] += vext_chunk.T @ PT               (rows 0-63 = out.T, 64 = l)
      epilogue, per 256-col half (starts before the block's last PV):
          DVE copies pv[0:65] -> bf16 SBUF, DMA to DRAM.
  - Host computes out = (pv_rows / l_row).T — the final normalize is part of
    the unshard/gather step (1M flops vs 17 GFLOP on device).
All matmul accumulation is fp32 (PSUM); bf16 operands give ~4e-3 l2 rel err.
"""

import numpy as np
import ml_dtypes
from contextlib import ExitStack

import concourse.bass as bass
from concourse import bacc
import concourse.mybir as mybir
import concourse.tile as tile
from concourse.bass import ts
from concourse.bass_utils import run_bass_kernel_spmd


B, T, C, H = 8, 2048, 1024, 64
P = 128
W_BLK = 512
HB = W_BLK // 2         # epilogue half-block
N_TB = T // W_BLK       # 4 t-blocks
N_C = C // P            # 8 contraction chunks
N_S = T // P            # 16 s-chunks
N_J = W_BLK // P        # 4 diagonal chunks per t-block
SCALE = float(H) ** -0.5
NEG = -1e30
HL = H + 1              # PV output rows: 64 out dims + 1 denominator row

MM_DT = mybir.dt.bfloat16
NP_MM = ml_dtypes.bfloat16
F32 = mybir.dt.float32

# consts tile layout (bf16 columns): [wqk_c | wv_c] per c-chunk, then ident.
# Per-chunk interleave so one small leading DMA covers the first proj chunks.
CH_W = 3 * H            # 192 cols per c-chunk (128 wqk + 64 wv)
ID_OFF = N_C * CH_W     # 1536
TRI_OFF = ID_OFF + H    # 1600: 0/1 lower-triangular mask (1 if t >= s)
CONST_W = TRI_OFF + P   # 1728


def build_nc() -> bacc.Bacc:
    nc = bacc.Bacc("TRN2")
    consts_d = nc.dram_tensor("consts", [P, CONST_W], MM_DT, kind="ExternalInput")
    # host pre-tiles xT so each [128, 512] tile is one contiguous 128KB read
    xT_d = nc.dram_tensor("xT", [N_TB, N_C, P, W_BLK], MM_DT, kind="ExternalInput")
    # unnormalized out rows 0-63 + denominator row 64, per t-block halves
    out_d = nc.dram_tensor("out", [HL, T], MM_DT, kind="ExternalOutput")

    with tile.TileContext(nc) as tc, ExitStack() as ctx:
        const = ctx.enter_context(tc.tile_pool(name="const", bufs=1))

        consts = const.tile([P, CONST_W], MM_DT)

        def wqk_c(c):
            return consts[:, c * CH_W: c * CH_W + 2 * H]

        def wv_c(c):
            return consts[:, c * CH_W + 2 * H: (c + 1) * CH_W]

        ident = consts[0:H, ID_OFF: ID_OFF + H]
        tri01 = consts[:, TRI_OFF: TRI_OFF + P]

        xt = {}

        def load_xt_pair(tb, c):
            # one DMA per 2 c-chunks: descriptor-gen on the SP queue
            # (~0.65us/instr) limits the feed rate, not the transfer
            t_ = const.tile([P, 2, W_BLK], MM_DT, name=f"xt{c}_{tb}")
            nc.sync.dma_start(
                t_, xT_d[tb, c:c + 2].rearrange("o p t -> p o t"))
            xt[(c, tb)] = t_[:, 0, :]
            xt[(c + 1, tb)] = t_[:, 1, :]

        # DMA order = first-use order: weight chunks 0-3 first, then the
        # first xt pair, then the rest
        nc.sync.dma_start(consts[:, 0:4 * CH_W], consts_d[:, 0:4 * CH_W])
        load_xt_pair(0, 0)
        nc.sync.dma_start(consts[:, 4 * CH_W:CONST_W],
                          consts_d[:, 4 * CH_W:CONST_W])
        load_xt_pair(0, 2)
        load_xt_pair(0, 4)
        load_xt_pair(0, 6)
        for tb in range(1, N_TB):
            for c in range(0, N_C, 2):
                load_xt_pair(tb, c)

        qT_blk = [const.tile([H, W_BLK], MM_DT, name=f"qT{tb}") for tb in range(N_TB)]
        kT_blk = [const.tile([H, W_BLK], MM_DT, name=f"kT{tb}") for tb in range(N_TB)]
        vT_blk = [const.tile([H, W_BLK], MM_DT, name=f"vT{tb}") for tb in range(N_TB)]
        # vext[s] = [v_nat(s) | ones-column]: PV yields out.T rows + l row
        vext = const.tile([P, N_S, HL], MM_DT, name="vext")
        nc.vector.memset(vext[:, :, H:HL], 1.0)

        with tc.tile_pool(name="ps_qk", bufs=1, space="PSUM") as ps_qk, \
             tc.tile_pool(name="ps_v", bufs=1, space="PSUM") as ps_v, \
             tc.tile_pool(name="ps_st", bufs=5, space="PSUM") as ps_st, \
             tc.tile_pool(name="ps_pv", bufs=1, space="PSUM") as ps_pv, \
             tc.tile_pool(name="ptp", bufs=7) as pt_pool, \
             tc.tile_pool(name="outp", bufs=4) as out_pool:

            # warm up the PE p-state during the DMA head: ~8 dummy matmuls on
            # garbage SBUF keep the PE continuously busy so the DVFS ramp
            # reaches full clock before the first real projection matmul.
            # Results land in st-pool tiles that real STs later overwrite
            # (start=True); the pool rotation orders them safely.
            warm_sb = const.tile([P, W_BLK], MM_DT, name="warm_sb")
            nc.vector.memset(warm_sb, 1.0)
            for wi in range(2):
                wtile = ps_st.tile([P, W_BLK], F32, tag="st", name=f"warm{wi}")
                for _ in range(4):
                    nc.tensor.matmul(wtile, warm_sb[:, 0:P], warm_sb[:],
                                     start=True, stop=True,
                                     skip_group_check=True)

            for tb in range(N_TB):
                # ---- proj(tb) ----
                qk_ps = ps_qk.tile([P, W_BLK], F32, tag="qk", name=f"qk{tb}")
                v_ps = ps_v.tile([H, W_BLK], F32, tag="v", name=f"v{tb}")
                # interleaved per c-chunk: each arriving xt tile feeds two
                # matmuls immediately, so the DMA stream keeps ahead of the PE
                for c in range(N_C):
                    nc.tensor.matmul(qk_ps, wqk_c(c), xt[(c, tb)],
                                     start=(c == 0), stop=(c == N_C - 1))
                    nc.tensor.matmul(v_ps, wv_c(c), xt[(c, tb)],
                                     start=(c == 0), stop=(c == N_C - 1))
                # vT first: the transpose chain (tr -> vext -> diag PV) is
                # longer than the qT/kT -> ST one; kT still lands before the
                # diagonal STs (5th+ in the ST stream) need it
                nc.vector.tensor_copy(vT_blk[tb][:], v_ps[:, :])
                nc.vector.tensor_copy(qT_blk[tb][:], qk_ps[0:H, :])
                # partition shift 64->0 (64-lane DVE op, quadrant-aligned)
                nc.vector.tensor_copy(kT_blk[tb][:], qk_ps[H:P, :])

                # ---- v transposes for this block (shares the qk psum tag) ----
                for j in range(N_J):
                    s = tb * N_J + j
                    tr = ps_qk.tile([P, H], MM_DT, tag="qk", name=f"tr{s}")
                    nc.tensor.transpose(tr, vT_blk[tb][:, ts(j, P)], ident)
                    nc.vector.tensor_copy(vext[:, s, 0:H], tr)

                # ---- attn(tb) ----
                pv = ps_pv.tile([HL, W_BLK], F32, tag="pv", name=f"pv{tb}")
                n_full = tb * N_J
                # (s_chunk, col offset within t-block, width)
                chunks = [(s, 0, W_BLK) for s in range(n_full)]
                chunks += [(n_full + j, j * P, W_BLK - j * P) for j in range(N_J)]
                n_ch = len(chunks)

                def emit_epi(half):
                    t0 = half * HB
                    ot = out_pool.tile([HL, HB], MM_DT, tag=f"ot{half}",
                                       name=f"ot{tb}_{half}")
                    nc.vector.tensor_copy(ot, pv[:, t0:t0 + HB])
                    nc.sync.dma_start(
                        out_d[:, tb * W_BLK + t0: tb * W_BLK + t0 + HB], ot)

                # single-chunk ST tiles (1 PSUM bank each) with lookahead-3
                # emission: PV(i) reaches the PE queue head only after the
                # exp(i) result is already in SBUF, so the PE pipeline never
                # drains on the exp dependency.
                pt_t = [None] * n_ch

                def emit_st(ci):
                    s, off, w = chunks[ci]
                    st_t = ps_st.tile([P, w], F32, tag="st", name=f"st{tb}_{ci}")
                    nc.tensor.matmul(st_t,
                                     kT_blk[s // N_J][:, ts(s % N_J, P)],
                                     qT_blk[tb][:, off:W_BLK],
                                     start=True, stop=True)
                    pt = pt_pool.tile([P, w], MM_DT, tag="pt",
                                      name=f"pt{tb}_{ci}")
                    nc.scalar.activation(pt, st_t,
                                         mybir.ActivationFunctionType.Exp,
                                         scale=SCALE)
                    if s >= n_full:
                        # diagonal: multiplicative 0/1 causal mask on the
                        # bf16 exp output (all-SBUF packed -> 4x DVE mode),
                        # off the ST->exp critical chain; exp of unmasked
                        # scores is finite so no inf*0 hazard
                        nc.vector.tensor_tensor(pt[:, 0:P], pt[:, 0:P],
                                                tri01, mybir.AluOpType.mult)
                    pt_t[ci] = pt

                def emit_pv(ci):
                    s, off, w = chunks[ci]
                    nc.tensor.matmul(pv[:, off:W_BLK], vext[:, s, :],
                                     pt_t[ci],
                                     start=(ci == 0), stop=(ci == n_ch - 1))

                LA = 4
                for ci in range(min(LA, n_ch)):
                    emit_st(ci)
                for ci in range(n_ch):
                    if ci + LA < n_ch:
                        emit_st(ci + LA)
                    emit_pv(ci)
                    # cols [0:256] are final once diag chunk j=1's PV is in
                    if ci == n_ch - 3:
                        emit_epi(0)
                emit_epi(1)

    nc.compile()
    return nc


_NC_CACHE = None


def _get_nc():
    global _NC_CACHE
    if _NC_CACHE is None:
        _NC_CACHE = build_nc()
    return _NC_CACHE


def prepare_in_maps(x, Wk, Wq, Wv):
    wqk = np.concatenate([np.asarray(Wq), np.asarray(Wk)], axis=1).astype(NP_MM)
    wv = np.asarray(Wv).astype(NP_MM)
    consts = np.zeros((P, CONST_W), dtype=NP_MM)
    wqk3 = wqk.reshape(N_C, P, 2 * H)
    wv3 = wv.reshape(N_C, P, H)
    for c in range(N_C):
        consts[:, c * CH_W: c * CH_W + 2 * H] = wqk3[c]
        consts[:, c * CH_W + 2 * H: (c + 1) * CH_W] = wv3[c]
    consts[0:H, ID_OFF:ID_OFF + H] = np.eye(H, dtype=NP_MM)
    ii = np.arange(P)
    consts[:, TRI_OFF:TRI_OFF + P] = (ii[None, :] >= ii[:, None]).astype(NP_MM)
    in_maps = []
    for b in range(B):
        xTb = np.asarray(x[b]).T.astype(NP_MM)  # [C, T]
        xT = np.ascontiguousarray(
            xTb.reshape(N_C, P, N_TB, W_BLK).transpose(2, 0, 1, 3)
        )  # [N_TB, N_C, 128, 512], each tile contiguous
        in_maps.append({"xT": xT, "consts": consts})
    return in_maps


def run(x, Wk, Wq, Wv, trace=False):
    nc = _get_nc()
    in_maps = prepare_in_maps(x, Wk, Wq, Wv)
    res = run_bass_kernel_spmd(nc, in_maps, core_ids=list(range(B)), trace=trace)
    outs = []
    for r in res.results:
        o = np.asarray(r["out"], dtype=np.float32)  # [65, T]
        outs.append((o[0:H, :] / o[H:HL, :]).T)     # normalize + transpose
    return np.stack(outs), res


def kernel(x, Wk, Wq, Wv):
    out, _ = run(x, Wk, Wq, Wv, trace=False)
    return out


# revision 43
# speedup vs baseline: 1.0129x; 1.0061x over previous
"""Single-head causal attention (B=8, T=2048, C=1024, H=64) on 8 TRN2 NeuronCores.

Strategy (data-parallel over batch, one batch element per core):
  - Host transposes x[b] -> xT [C, T], casts matmul operands to bf16, and
    prepacks all weights/constants into one SBUF-layout buffer.  DMA order
    is by first use (wqk chunk 0, xt tile 0, remaining wqk, ...) so the
    first proj matmul starts ~2us earlier than with per-tensor DMAs.
  - Device, per core, per 512-wide t-block tb:
      proj(tb):  qT,kT = ([Wq|Wk].T @ xT_tb) packed in one PE pass; vT = Wv.T @ xT_tb
      evac(tb):  PSUM -> SBUF bf16 casts (kT via 64->0 partition-shift DVE copy)
      trans(tb): v chunks rebuilt in natural [s, h] layout via PE transpose,
                 with ONE ones-column appended (vext, [128, 65]) so the PV
                 matmul also produces the softmax denominator l in row 64.
                 65 output partitions instead of 128 halve the PE array
                 energy of the PV pass (the chip power-throttles the PE when
                 sustained power is too high, so energy == time; GpSimd is
                 kept idle for the same reason).
      attn(tb), per s-chunk pair (causally trimmed, exact packed widths):
          ST[s, t] = kT_chunk.T @ qT_block            (PSUM, <=2 banks/pair)
          diag chunks: += causal additive mask on first 128 cols (DVE)
          PT = exp(SCALE * ST)                        (one ACT per pair, bf16 out)
          PV[:, t] += vext_chunk.T @ PT               (rows 0-63 = out.T, 64 = l)
      epilogue, per 256-col half (starts before the block's last PV):
          DVE copies pv[0:65] -> bf16 SBUF, DMA to DRAM.
  - Host computes out = (pv_rows / l_row).T — the final normalize is part of
    the unshard/gather step (1M flops vs 17 GFLOP on device).
All matmul accumulation is fp32 (PSUM); bf16 operands give ~4e-3 l2 rel err.
"""

import numpy as np
import ml_dtypes
from contextlib import ExitStack

import concourse.bass as bass
from concourse import bacc
import concourse.mybir as mybir
import concourse.tile as tile
from concourse.bass import ts
from concourse.bass_utils import run_bass_kernel_spmd


B, T, C, H = 8, 2048, 1024, 64
P = 128
W_BLK = 512
HB = W_BLK // 2         # epilogue half-block
N_TB = T // W_BLK       # 4 t-blocks
N_C = C // P            # 8 contraction chunks
N_S = T // P            # 16 s-chunks
N_J = W_BLK // P        # 4 diagonal chunks per t-block
SCALE = float(H) ** -0.5
NEG = -1e30
HL = H + 1              # PV output rows: 64 out dims + 1 denominator row

MM_DT = mybir.dt.bfloat16
NP_MM = ml_dtypes.bfloat16
F32 = mybir.dt.float32

# consts tile layout (bf16 columns): [wqk_c | wv_c] per c-chunk, then ident.
# Per-chunk interleave so one small leading DMA covers the first proj chunks.
CH_W = 3 * H            # 192 cols per c-chunk (128 wqk + 64 wv)
ID_OFF = N_C * CH_W     # 1536
TRI_OFF = ID_OFF + H    # 1600: 0/1 lower-triangular mask (1 if t >= s)
CONST_W = TRI_OFF + P   # 1728


def build_nc() -> bacc.Bacc:
    nc = bacc.Bacc("TRN2")
    consts_d = nc.dram_tensor("consts", [P, CONST_W], MM_DT, kind="ExternalInput")
    # host pre-tiles xT so each [128, 512] tile is one contiguous 128KB read
    xT_d = nc.dram_tensor("xT", [N_TB, N_C, P, W_BLK], MM_DT, kind="ExternalInput")
    # unnormalized out rows 0-63 + denominator row 64, per t-block halves
    out_d = nc.dram_tensor("out", [HL, T], MM_DT, kind="ExternalOutput")

    with tile.TileContext(nc) as tc, ExitStack() as ctx:
        const = ctx.enter_context(tc.tile_pool(name="const", bufs=1))

        consts = const.tile([P, CONST_W], MM_DT)

        def wqk_c(c):
            return consts[:, c * CH_W: c * CH_W + 2 * H]

        def wv_c(c):
            return consts[:, c * CH_W + 2 * H: (c + 1) * CH_W]

        ident = consts[0:H, ID_OFF: ID_OFF + H]
        tri01 = consts[:, TRI_OFF: TRI_OFF + P]

        xt = {}

        def load_xt_pair(tb, c):
            # one DMA per 2 c-chunks: descriptor-gen on the SP queue
            # (~0.65us/instr) limits the feed rate, not the transfer
            t_ = const.tile([P, 2, W_BLK], MM_DT, name=f"xt{c}_{tb}")
            nc.sync.dma_start(
                t_, xT_d[tb, c:c + 2].rearrange("o p t -> p o t"))
            xt[(c, tb)] = t_[:, 0, :]
            xt[(c + 1, tb)] = t_[:, 1, :]

        # DMA order = first-use order: weight chunks 0-3 first, then the
        # first xt pair, then the rest
        nc.sync.dma_start(consts[:, 0:4 * CH_W], consts_d[:, 0:4 * CH_W])
        load_xt_pair(0, 0)
        nc.sync.dma_start(consts[:, 4 * CH_W:CONST_W],
                          consts_d[:, 4 * CH_W:CONST_W])
        load_xt_pair(0, 2)
        load_xt_pair(0, 4)
        load_xt_pair(0, 6)
        for tb in range(1, N_TB):
            for c in range(0, N_C, 2):
                load_xt_pair(tb, c)

        qT_blk = [const.tile([H, W_BLK], MM_DT, name=f"qT{tb}") for tb in range(N_TB)]
        kT_blk = [const.tile([H, W_BLK], MM_DT, name=f"kT{tb}") for tb in range(N_TB)]
        vT_blk = [const.tile([H, W_BLK], MM_DT, name=f"vT{tb}") for tb in range(N_TB)]
        # vext[s] = [v_nat(s) | ones-column]: PV yields out.T rows + l row
        vext = const.tile([P, N_S, HL], MM_DT, name="vext")
        nc.vector.memset(vext[:, :, H:HL], 1.0)

        with tc.tile_pool(name="ps_qk", bufs=1, space="PSUM") as ps_qk, \
             tc.tile_pool(name="ps_v", bufs=1, space="PSUM") as ps_v, \
             tc.tile_pool(name="ps_st", bufs=5, space="PSUM") as ps_st, \
             tc.tile_pool(name="ps_pv", bufs=1, space="PSUM") as ps_pv, \
             tc.tile_pool(name="ptp", bufs=7) as pt_pool, \
             tc.tile_pool(name="outp", bufs=4) as out_pool:

            # warm up the PE p-state during the DMA head: ~8 dummy matmuls on
            # garbage SBUF keep the PE continuously busy so the DVFS ramp
            # reaches full clock before the first real projection matmul.
            # Results land in st-pool tiles that real STs later overwrite
            # (start=True); the pool rotation orders them safely.
            warm_sb = const.tile([P, W_BLK], MM_DT, name="warm_sb")
            nc.vector.memset(warm_sb, 1.0)
            for wi in range(2):
                wtile = ps_st.tile([P, W_BLK], F32, tag="st", name=f"warm{wi}")
                for _ in range(4):
                    nc.tensor.matmul(wtile, warm_sb[:, 0:P], warm_sb[:],
                                     start=True, stop=True,
                                     skip_group_check=True)

            for tb in range(N_TB):
                # ---- proj(tb) ----
                qk_ps = ps_qk.tile([P, W_BLK], F32, tag="qk", name=f"qk{tb}")
                v_ps = ps_v.tile([H, W_BLK], F32, tag="v", name=f"v{tb}")
                # interleaved per c-chunk: each arriving xt tile feeds two
                # matmuls immediately, so the DMA stream keeps ahead of the PE
                for c in range(N_C):
                    nc.tensor.matmul(qk_ps, wqk_c(c), xt[(c, tb)],
                                     start=(c == 0), stop=(c == N_C - 1))
                    nc.tensor.matmul(v_ps, wv_c(c), xt[(c, tb)],
                                     start=(c == 0), stop=(c == N_C - 1))
                # vT first: the transpose chain (tr -> vext -> diag PV) is
                # longer than the qT/kT -> ST one; kT still lands before the
                # diagonal STs (5th+ in the ST stream) need it
                nc.vector.tensor_copy(vT_blk[tb][:], v_ps[:, :])
                nc.vector.tensor_copy(qT_blk[tb][:], qk_ps[0:H, :])
                # partition shift 64->0 (64-lane DVE op, quadrant-aligned)
                nc.vector.tensor_copy(kT_blk[tb][:], qk_ps[H:P, :])

                # ---- v transposes for this block (shares the qk psum tag) ----
                for j in range(N_J):
                    s = tb * N_J + j
                    tr = ps_qk.tile([P, H], MM_DT, tag="qk", name=f"tr{s}")
                    nc.tensor.transpose(tr, vT_blk[tb][:, ts(j, P)], ident)
                    nc.vector.tensor_copy(vext[:, s, 0:H], tr)

                # ---- attn(tb) ----
                pv = ps_pv.tile([HL, W_BLK], F32, tag="pv", name=f"pv{tb}")
                n_full = tb * N_J
                # (s_chunk, col offset within t-block, width)
                chunks = [(s, 0, W_BLK) for s in range(n_full)]
                chunks += [(n_full + j, j * P, W_BLK - j * P) for j in range(N_J)]
                n_ch = len(chunks)

                def emit_epi(half):
                    t0 = half * HB
                    ot = out_pool.tile([HL, HB], MM_DT, tag=f"ot{half}",
                                       name=f"ot{tb}_{half}")
                    nc.vector.tensor_copy(ot, pv[:, t0:t0 + HB])
                    nc.sync.dma_start(
                        out_d[:, tb * W_BLK + t0: tb * W_BLK + t0 + HB], ot)

                # single-chunk ST tiles (1 PSUM bank each) with lookahead-3
                # emission: PV(i) reaches the PE queue head only after the
                # exp(i) result is already in SBUF, so the PE pipeline never
                # drains on the exp dependency.
                pt_t = [None] * n_ch

                def emit_st(ci):
                    s, off, w = chunks[ci]
                    st_t = ps_st.tile([P, w], F32, tag="st", name=f"st{tb}_{ci}")
                    nc.tensor.matmul(st_t,
                                     kT_blk[s // N_J][:, ts(s % N_J, P)],
                                     qT_blk[tb][:, off:W_BLK],
                                     start=True, stop=True)
                    pt = pt_pool.tile([P, w], MM_DT, tag="pt",
                                      name=f"pt{tb}_{ci}")
                    nc.scalar.activation(pt, st_t,
                                         mybir.ActivationFunctionType.Exp,
                                         scale=SCALE)
                    if s >= n_full:
                        # diagonal: multiplicative 0/1 causal mask on the
                        # bf16 exp output (all-SBUF packed -> 4x DVE mode),
                        # off the ST->exp critical chain; exp of unmasked
                        # scores is finite so no inf*0 hazard
                        nc.vector.tensor_tensor(pt[:, 0:P], pt[:, 0:P],
                                                tri01, mybir.AluOpType.mult)
                    pt_t[ci] = pt

                def emit_pv(ci):
                    s, off, w = chunks[ci]
                    nc.tensor.matmul(pv[:, off:W_BLK], vext[:, s, :],
                                     pt_t[ci],
                                     start=(ci == 0), stop=(ci == n_ch - 1))

                LA = 4
                for ci in range(min(LA, n_ch)):
                    emit_st(ci)
                for ci in range(n_ch):
                    if ci + LA < n_ch:
                        emit_st(ci + LA)
                    emit_pv(ci)
                    # cols [0:256] are final once diag chunk j=1's PV is in
                    if ci == n_ch - 3:
                        emit_epi(0)
                emit_epi(1)

    nc.compile()
    return nc


_NC_CACHE = None


def _get_nc():
    global _NC_CACHE
    if _NC_CACHE is None:
        _NC_CACHE = build_nc()
    return _NC_CACHE


def prepare_in_maps(x, Wk, Wq, Wv):
    wqk = np.concatenate([np.asarray(Wq), np.asarray(Wk)], axis=1).astype(NP_MM)
    wv = np.asarray(Wv).astype(NP_MM)
    consts = np.zeros((P, CONST_W), dtype=NP_MM)
    wqk3 = wqk.reshape(N_C, P, 2 * H)
    wv3 = wv.reshape(N_C, P, H)
    for c in range(N_C):
        consts[:, c * CH_W: c * CH_W + 2 * H] = wqk3[c]
        consts[:, c * CH_W + 2 * H: (c + 1) * CH_W] = wv3[c]
    consts[0:H, ID_OFF:ID_OFF + H] = np.eye(H, dtype=NP_MM)
    ii = np.arange(P)
    consts[:, TRI_OFF:TRI_OFF + P] = (ii[None, :] >= ii[:, None]).astype(NP_MM)
    in_maps = []
    for b in range(B):
        xTb = np.asarray(x[b]).T.astype(NP_MM)  # [C, T]
        xT = np.ascontiguousarray(
            xTb.reshape(N_C, P, N_TB, W_BLK).transpose(2, 0, 1, 3)
        )  # [N_TB, N_C, 128, 512], each tile contiguous
        in_maps.append({"xT": xT, "consts": consts})
    return in_maps


def run(x, Wk, Wq, Wv, trace=False):
    nc = _get_nc()
    in_maps = prepare_in_maps(x, Wk, Wq, Wv)
    res = run_bass_kernel_spmd(nc, in_maps, core_ids=list(range(B)), trace=trace)
    outs = []
    for r in res.results:
        o = np.asarray(r["out"], dtype=np.float32)  # [65, T]
        outs.append((o[0:H, :] / o[H:HL, :]).T)     # normalize + transpose
    return np.stack(outs), res


def kernel(x, Wk, Wq, Wv):
    out, _ = run(x, Wk, Wq, Wv, trace=False)
    return out
